# revision 5
# baseline (speedup 1.0000x reference)
"""Data-parallel Trainium kernel for nn_Net_55774445305918 (loss_fn).

Strategy (per sharding_hint): pure data parallel over the batch dim of
first/first_prev/second across 8 NeuronCores; conv/BN params replicated.
BatchNorm uses full-batch statistics, so the encoder is split into two
device phases with a tiny host-side cross-shard stat combine in between
(the "psum-style cross-device mean"):

  phase1: conv1+relu, conv2  -> h kept on-device, per-shard sum/sumsq out
  host:   combine [8,30] partial sums -> full-batch mu/var (f64 accum)
  phase2: BN+relu, conv3, bilinear resize, spatial softmax, keypoints,
          overlap einsum vs img_change

Losses (kc, sc, kv) are cheap [B,K]-sized reductions done on host in f64,
faithful to the reference (including the J=2 keypoint-variety indexing
quirk and the swapped col/row keypoint order).
"""
import numpy as np
import jax
import jax.numpy as jnp
from functools import partial

BN_EPS = 1e-5
LOSS_EPS = 1e-7
HW = 80
B, CIN, K = 512, 3, 3
NDEV = 8
BS = B // NDEV  # 64 images per core


def _conv2d(x, w, b, stride):
    out = jax.lax.conv_general_dilated(
        x, w, window_strides=(stride, stride), padding='VALID',
        dimension_numbers=('NCHW', 'OIHW', 'NCHW'))
    return out + b[None, :, None, None]


def _phase1(x, w1, b1, w2, b2):
    # x: [BS,3,80,80] -> h: [BS,30,38,38], plus BN stat partials
    x2 = jax.nn.relu(_conv2d(x - 0.33, w1, b1, 1))
    h = _conv2d(x2, w2, b2, 2)
    s1 = jnp.sum(h, axis=(0, 2, 3))
    s2 = jnp.sum(h * h, axis=(0, 2, 3))
    return h, s1, s2


_RESIZE_R = None


def _resize_mat():
    # Exact 18->80 bilinear-resize matrix, extracted from jax.image.resize
    # itself (resize is linear, so columns are resize(e_i)).
    global _RESIZE_R
    if _RESIZE_R is None:
        with jax.default_device(jax.devices('cpu')[0]):
            eye = np.eye(18, dtype=np.float32)
            R = jax.image.resize(eye, (18, HW), method='bilinear')  # rows resized? no: shape (18,80) resizes axis1
            _RESIZE_R = np.asarray(R, np.float32).T  # [80, 18] acting on axis of len 18
    return _RESIZE_R


def _phase2(h, mu, var, gamma, beta, w3, b3, chg, R):
    # h: [BS,30,38,38]; mu/var: [30] full-batch stats; chg: [BS,80,80]
    x3 = jax.nn.relu((h - mu[None, :, None, None])
                     * jax.lax.rsqrt(var[None, :, None, None] + BN_EPS)
                     * gamma[None, :, None, None] + beta[None, :, None, None])
    dense = _conv2d(x3, w3, b3, 2)                       # [BS,K,18,18]
    # separable bilinear resize as two dense contractions (matmul-friendly)
    up = jnp.einsum('oi,bkij,pj->bkop', R, dense, R)     # [BS,K,80,80]
    soft = jax.nn.softmax(up.reshape(-1, K, HW * HW), axis=-1).reshape(-1, K, HW, HW)
    idx = jnp.arange(HW, dtype=jnp.float32)
    col = jnp.sum(soft * idx[None, None, None, :], axis=(2, 3))
    row = jnp.sum(soft * idx[None, None, :, None], axis=(2, 3))
    xy = jnp.stack([col, row], axis=2)                   # [BS,K,2]
    overlap = jnp.einsum('bkhw,bhw->bk', soft, chg)      # [BS,K]
    return xy, soft, overlap


def _chg_fn(first, second):
    return jnp.sum((jnp.abs(first - second) > 0).astype(jnp.float32), axis=1)


_P1 = None
_P2 = None
_PC = None


def _get_pmapped():
    global _P1, _P2, _PC
    if _P1 is None:
        _P1 = jax.pmap(_phase1)
        _P2 = jax.pmap(_phase2)
        _PC = jax.pmap(_chg_fn)
    return _P1, _P2, _PC


def _shard(x):
    return x.reshape((NDEV, BS) + x.shape[1:])


def _rep(x):
    return np.broadcast_to(x, (NDEV,) + x.shape)


_DEVICE_OK = [True]


def _device_kernel(first, first_prev, second, w1, b1, w2, b2, gamma, beta, w3, b3):
    p1, p2, pc = _get_pmapped()
    first = np.asarray(first, np.float32)
    first_prev = np.asarray(first_prev, np.float32)
    second = np.asarray(second, np.float32)
    params1 = [_rep(np.asarray(a, np.float32)) for a in (w1, b1, w2, b2)]
    gamma_r, beta_r, w3_r, b3_r = (_rep(np.asarray(a, np.float32))
                                   for a in (gamma, beta, w3, b3))

    R_r = _rep(_resize_mat())
    f_sh = _shard(first)
    chg = pc(f_sh, _shard(second))                       # [8,BS,80,80] on-device

    n_total = np.float64(B * 38 * 38)
    outs = {}
    for name, x in (('kp1', f_sh), ('kp1_prev', _shard(first_prev)),
                    ('kp2', _shard(second))):
        h, s1, s2 = p1(x, *params1)
        s1 = np.asarray(s1, np.float64).sum(0)
        s2 = np.asarray(s2, np.float64).sum(0)
        mu = s1 / n_total
        var = s2 / n_total - mu * mu
        mu_r = _rep(mu.astype(np.float32))
        var_r = _rep(var.astype(np.float32))
        xy, soft, overlap = p2(h, mu_r, var_r, gamma_r, beta_r, w3_r, b3_r, chg, R_r)
        outs[name] = (xy, soft, overlap)

    kp1, map1, overlap1 = outs['kp1']
    kp1_prev, _, _ = outs['kp1_prev']
    kp2, map2, _ = outs['kp2']

    kp1 = np.asarray(kp1, np.float32).reshape(B, K, 2)
    kp2 = np.asarray(kp2, np.float32).reshape(B, K, 2)
    kp1_prev = np.asarray(kp1_prev, np.float32).reshape(B, K, 2)
    map1 = np.asarray(map1, np.float32).reshape(B, K, HW, HW)
    map2 = np.asarray(map2, np.float32).reshape(B, K, HW, HW)
    img_change = np.asarray(chg, np.float32).reshape(B, HW, HW)
    overlap1 = np.asarray(overlap1, np.float64).reshape(B, K)

    # --- losses on host (tiny, f64 accum, cast back to f32) ---
    kp1d = kp1.astype(np.float64)
    kp1pd = kp1_prev.astype(np.float64)
    kc = np.mean(np.sum(kp1d - kp1pd, axis=2) ** 2)

    s = -np.log(LOSS_EPS + overlap1)                     # [B,K]
    mask = (img_change.astype(np.float64).mean(axis=(1, 2)) > 0).astype(np.float64)
    sc = np.sum(s * mask[:, None]) / max(mask.sum() * K, 1.0)

    d = np.sum((kp1d[:, :, None, :] - kp1d[:, None, :2, :]) ** 2, axis=-1)
    ij_mask = (np.arange(K)[:, None] != np.arange(2)[None, :]).astype(np.float64)
    kv = np.sum(np.maximum(d, 0.0) * ij_mask[None]) / (K * K * B)

    return (kp1, kp2, kp1_prev, map1, map2, img_change,
            np.float32(kv), np.float32(kc), np.float32(sc))


def _host_kernel(first, first_prev, second, w1, b1, w2, b2, gamma, beta, w3, b3):
    """Same math on jax-CPU (fallback when the neuron compile fails)."""
    cpu = jax.devices('cpu')[0]
    with jax.default_device(cpu):
        args = [jnp.asarray(np.asarray(a, np.float32)) for a in
                (first, first_prev, second, w1, b1, w2, b2, gamma, beta, w3, b3)]
        first, first_prev, second, w1, b1, w2, b2, gamma, beta, w3, b3 = args
        R = jnp.asarray(_resize_mat())
        p1j = jax.jit(_phase1)
        p2j = jax.jit(_phase2)
        chg = jax.jit(_chg_fn)(first, second)

        def encode(x):
            h, s1, s2 = p1j(x, w1, b1, w2, b2)
            n = np.float64(x.shape[0] * 38 * 38)
            mu = (np.asarray(s1, np.float64) / n)
            var = np.asarray(s2, np.float64) / n - mu * mu
            return p2j(h, jnp.asarray(mu, jnp.float32),
                       jnp.asarray(var, jnp.float32),
                       gamma, beta, w3, b3, chg, R)

        kp1, map1, overlap1 = encode(first)
        kp1_prev, _, _ = encode(first_prev)
        kp2, map2, _ = encode(second)

    kp1 = np.asarray(kp1, np.float32)
    kp2 = np.asarray(kp2, np.float32)
    kp1_prev = np.asarray(kp1_prev, np.float32)
    map1 = np.asarray(map1, np.float32)
    map2 = np.asarray(map2, np.float32)
    img_change = np.asarray(chg, np.float32)
    overlap1 = np.asarray(overlap1, np.float64)

    kp1d = kp1.astype(np.float64)
    kp1pd = kp1_prev.astype(np.float64)
    kc = np.mean(np.sum(kp1d - kp1pd, axis=2) ** 2)
    s = -np.log(LOSS_EPS + overlap1)
    mask = (img_change.astype(np.float64).mean(axis=(1, 2)) > 0).astype(np.float64)
    sc = np.sum(s * mask[:, None]) / max(mask.sum() * K, 1.0)
    d = np.sum((kp1d[:, :, None, :] - kp1d[:, None, :2, :]) ** 2, axis=-1)
    ij_mask = (np.arange(K)[:, None] != np.arange(2)[None, :]).astype(np.float64)
    kv = np.sum(np.maximum(d, 0.0) * ij_mask[None]) / (K * K * B)

    return (kp1, kp2, kp1_prev, map1, map2, img_change,
            np.float32(kv), np.float32(kc), np.float32(sc))


def kernel(**inputs):
    if _DEVICE_OK[0]:
        try:
            return _device_kernel(**inputs)
        except Exception as e:  # neuronx-cc internal errors, device issues
            import sys
            print(f"kernel: device path failed ({type(e).__name__}); "
                  f"falling back to host execution", file=sys.stderr)
            _DEVICE_OK[0] = False
    return _host_kernel(**inputs)
